# revision 61
# baseline (speedup 1.0000x reference)
"""Causal self-attention TRN2 kernel (B=2, L=2048, D=1024, H=16, dh=64).

Sharding: 8 cores = 2 batches x 4 head-groups. Core c handles batch c//4 and
heads [4g..4g+3] where g = c % 4, as two head-pairs ("units").

All device data is fp16 (converted on host); PSUM accumulation stays fp32.

Per-core device program (SPMD, same program all cores, different data):
  phase 1: qT/kT = W.T @ x per unit ([dh-pair 128, L]); vT likewise, then
           PE-transposed into V chunks laid out [128 k, (j,h,65)] where
           col 64 of each 65-wide chunk is a ones column (row-sum rider).
  phase 2: per unit, per 512-wide q-block I:
           S^T chunks [128k x 512q] (fp16 in, f32 PSUM), exp on ScalarE
           (fp16 out, scale=1/8, no max-sub), triangular mask multiplies
           on the 128-wide diagonal chunk only, then AV with P^T chunks as
           the STATIONARY operand and [V|1] as moving: out [128 q, 65]
           accumulated over j; col 64 = row sums l. Normalize via DVE
           reciprocal + per-partition tensor_scalar, then PE-transpose
           O [q, dh] -> O^T [dh, q] for the projection.
  phase 3: y_partial[q, :] = outT.T @ w_out_local, fp16 out, DMA to DRAM.
Host: y[b] = sum of the 4 partial outputs for batch b (fp32 accumulate).
"""

import numpy as np

import concourse.bass as bass
import concourse.mybir as mybir
from concourse import bacc
import concourse.tile as tile
from concourse.bass_utils import run_bass_kernel_spmd

F32 = mybir.dt.float32
FP16 = mybir.dt.float16
EXP = mybir.ActivationFunctionType.Exp

B, L, D = 2, 2048, 1024
H, DH = 16, 64
NCORES = 8
NQB = L // 512          # q-blocks per sequence (4)

_CACHE = {}
LAST_RESULT = None      # BassKernelResults of the most recent run (for test.py)


def _build():
    nc = bacc.Bacc("TRN2", target_bir_lowering=False, debug=False,
                   num_devices=NCORES)

    xt_d = nc.dram_tensor("xt", [D, L], FP16, kind="ExternalInput").ap()
    wqkv_d = nc.dram_tensor("wqkv", [D, 768], FP16, kind="ExternalInput").ap()
    wout_d = nc.dram_tensor("wout", [2, 128, 1024], FP16,
                            kind="ExternalInput").ap()
    consts_d = nc.dram_tensor("consts", [128, 320], FP16,
                              kind="ExternalInput").ap()
    y_d = nc.dram_tensor("y", [L, D], FP16, kind="ExternalOutput").ap()

    with tile.TileContext(nc) as tc:
        with tc.tile_pool(name="persist", bufs=1) as pp, \
             tc.tile_pool(name="work", bufs=1) as wp, \
             tc.tile_pool(name="psstage", bufs=2, space="PSUM") as ps_stage, \
             tc.tile_pool(name="psacc", bufs=2, space="PSUM") as ps_acc, \
             tc.tile_pool(name="psav", bufs=2, space="PSUM") as ps_av:

            # ---- persistent SBUF tiles ----
            wq = pp.tile([128, 8 * 768], FP16, name="wq")
            wq_v = wq.rearrange("p (d c) -> p d c", d=8)
            wq_src = wqkv_d.rearrange("(a b) c -> b a c", b=128)
            xt = pp.tile([128, 8 * L], FP16, name="xt")
            xt_v = xt.rearrange("p (d l) -> p d l", d=8)
            xt_src = xt_d.rearrange("(a b) c -> b a c", b=128)
            consts = pp.tile([128, 320], FP16, name="consts")
            eye64 = consts[:, 0:64]      # eye(64) stacked twice on partitions
            eye128 = consts[:, 64:192]
            trimask = consts[:, 192:320]  # [k, q] = 1 if k <= q
            wout = pp.tile([128, 2048], FP16, name="wout")
            wout_v = wout.rearrange("p (u n) -> p u n", u=2)

            # ---- input DMAs (order = SP issue order) ----
            def load_wq(u, dh):
                ds_ = slice(dh * 4, (dh + 1) * 4)
                cs = slice(u * 384, (u + 1) * 384)
                nc.sync.dma_start(out=wq_v[:, ds_, cs], in_=wq_src[:, ds_, cs])

            def load_xt(q, dh):
                ds_ = slice(dh * 4, (dh + 1) * 4)
                nc.sync.dma_start(
                    out=xt_v[:, ds_, q * 512:(q + 1) * 512],
                    in_=xt_src[:, ds_, q * 512:(q + 1) * 512])

            load_wq(0, 0)
            nc.sync.dma_start(out=xt_v[:, 0:4, 0:256],
                              in_=xt_src[:, 0:4, 0:256])
            load_wq(0, 1)
            nc.sync.dma_start(out=xt_v[:, 4:8, 0:256],
                              in_=xt_src[:, 4:8, 0:256])
            nc.sync.dma_start(out=xt_v[:, 0:4, 256:512],
                              in_=xt_src[:, 0:4, 256:512])
            nc.sync.dma_start(out=xt_v[:, 4:8, 256:512],
                              in_=xt_src[:, 4:8, 256:512])
            nc.sync.dma_start(out=consts, in_=consts_d)
            load_xt(1, 0)
            load_xt(1, 1)
            load_xt(2, 0)
            load_xt(2, 1)
            load_xt(3, 0)
            load_xt(3, 1)
            load_wq(1, 0)
            load_wq(1, 1)
            nc.sync.dma_start(out=wout,
                              in_=wout_d.rearrange("a b c -> b a c"))

            # per-unit persistent tensors
            qT = [pp.tile([128, L], FP16, name=f"qT{u}") for u in range(2)]
            kT = [pp.tile([128, L], FP16, name=f"kT{u}") for u in range(2)]
            vTb = [pp.tile([128, L], FP16, name=f"vTb{u}") for u in range(2)]
            # V chunks: [128 k, (j 16, h 2, 65)]; col 64 of each chunk = 1.0
            vsb = [pp.tile([128, 16 * 2 * 65], FP16, name=f"vsb{u}")
                   for u in range(2)]
            vsb_v = [vsb[u].rearrange("p (j h c) -> p j h c", j=16, h=2)
                     for u in range(2)]
            # normalized attention output O [q, (c 16, h 2, 64)]
            outSB = [pp.tile([128, 2048], FP16, name=f"oS{u}")
                     for u in range(2)]
            outSB_v = [outSB[u].rearrange("p (c h e) -> p c h e", c=16, h=2)
                       for u in range(2)]
            # O^T [dh-pair 128, q 2048]
            outT = [pp.tile([128, L], FP16, name=f"outT{u}") for u in range(2)]

            for u in range(2):
                nc.gpsimd.memset(vsb_v[u][:, :, :, 64:65], 1.0)

            COPY = mybir.ActivationFunctionType.Copy

            def qkv_pass(u, r):
                # q/k/v projections for token window r (512 tokens), then
                # the V transposes for that window's 4 k-chunks.
                for ci, dest in ((0, qT[u]), (1, kT[u]), (2, vTb[u])):
                    cof = u * 384 + ci * 128
                    acc = ps_acc.tile([128, 512], F32,
                                      name=f"qkv{u}{ci}{r}", tag="acc")
                    if u == 0 and r == 0:
                        # first window: half-width groups so the leading
                        # matmuls only need the first quarter-window DMA
                        for half in range(2):
                            hs = slice(half * 256, (half + 1) * 256)
                            for d in range(8):
                                nc.tensor.matmul(
                                    acc[:, hs],
                                    wq[:, d * 768 + cof:d * 768 + cof + 128],
                                    xt[:, d * L + half * 256:
                                       d * L + half * 256 + 256],
                                    start=(half == 0 and d == 0),
                                    stop=(d == 7),
                                    skip_group_check=True)
                        for d in []:
                            pass
                    else:
                        for d in range(8):
                            nc.tensor.matmul(
                                acc,
                                wq[:, d * 768 + cof:d * 768 + cof + 128],
                                xt[:, d * L + r * 512:d * L + (r + 1) * 512],
                                start=(d == 0), stop=(d == 7),
                                skip_group_check=True)
                    nc.vector.tensor_copy(
                        dest[:, r * 512:(r + 1) * 512], acc)
                # V^T -> V transposes for chunks j = 4r..4r+3, both heads
                for h2 in range(2):
                    st = ps_stage.tile([128, 256], FP16, name=f"vt{u}{r}{h2}",
                                       tag="stage")
                    for jj in range(4):
                        j = 4 * r + jj
                        nc.tensor.transpose(
                            st[:, jj * 64:(jj + 1) * 64],
                            vTb[u][h2 * 64:(h2 + 1) * 64,
                                   j * 128:(j + 1) * 128],
                            eye64[h2 * 64:(h2 + 1) * 64, :])
                    nc.vector.tensor_copy(
                        vsb_v[u][:, 4 * r:4 * r + 4, h2, 0:64],
                        st.rearrange("p (j e) -> p j e", j=4))

            def attn_qblock(u, I):
                nj = 4 * (I + 1)
                avp = [ps_av.tile([128, 260], F32, name=f"av{u}{I}{h}",
                                  tag="av") for h in range(2)]
                qs = I * 512
                for j in range(nj):
                    ks = slice(j * 128, (j + 1) * 128)
                    m = j - 4 * I
                    qlo = max(m, 0) * 128   # first valid q_rel
                    st = ps_stage.tile([128, 1024], F32,
                                       name=f"st{u}{I}{j}", tag="stage")
                    nc.tensor.matmul(st[:, qlo:512], kT[u][0:64, ks],
                                     qT[u][0:64, qs + qlo:qs + 512],
                                     start=True, stop=True,
                                     skip_group_check=True)
                    nc.tensor.matmul(st[:, 512 + qlo:1024],
                                     kT[u][64:128, ks],
                                     qT[u][64:128, qs + qlo:qs + 512],
                                     start=True, stop=True,
                                     skip_group_check=True)
                    pt = wp.tile([128, 1024], FP16, name=f"pt{u}{I}{j}",
                                 tag="pt", bufs=5)
                    if qlo == 0:
                        nc.scalar.activation(pt, st, EXP, scale=0.125)
                    else:
                        pt_v = pt.rearrange("p (h w) -> p h w", h=2)
                        st_v = st.rearrange("p (h w) -> p h w", h=2)
                        nc.scalar.activation(pt_v[:, :, qlo:512],
                                             st_v[:, :, qlo:512], EXP,
                                             scale=0.125)
                    if m >= 0:
                        # triangular mask on the diagonal 128-wide chunk only
                        # (SBUF-only op, so it can live on GpSimd/Pool)
                        pt_m = pt.rearrange("p (h w) -> p h w", h=2)
                        tri2 = trimask.rearrange("p (o w) -> p o w", o=1)
                        nc.vector.tensor_tensor(
                            pt_m[:, :, qlo:qlo + 128],
                            pt_m[:, :, qlo:qlo + 128],
                            tri2.broadcast_to((128, 2, 128)),
                            op=mybir.AluOpType.mult)
                    # AV: P^T chunk stationary, [V|1] moving -> [q, 65]
                    # start=True clears has_written for the WHOLE bank, so
                    # only the very first matmul into each av tile may set it;
                    # the other groups' first writes land on cleared bits and
                    # overwrite (set-bit) per element, then accumulate.
                    for h in range(2):
                        for c in range(max(m, 0), 4):
                            nc.tensor.matmul(
                                avp[h][:, c * 65:(c + 1) * 65],
                                pt[:, h * 512 + c * 128:
                                   h * 512 + (c + 1) * 128],
                                vsb_v[u][:, j, h, :],
                                start=(j == 0 and c == 0),
                                stop=(j == 4 * I + c),
                                skip_group_check=True)
                # normalize: per-partition 1/l (l rides in col 64 of chunks)
                for h in range(2):
                    av_v = avp[h].rearrange("p (c e) -> p c e", e=65)
                    rl = wp.tile([128, 4], F32, name=f"rl{u}{I}{h}",
                                 tag="rl", bufs=4)
                    nc.vector.reciprocal(rl, av_v[:, :, 64:65])
                    for c in range(4):
                        if I >= 2 and h == 0:
                            nc.scalar.activation(
                                outSB_v[u][:, I * 4 + c, h, :],
                                av_v[:, c, 0:64], COPY,
                                scale=rl[:, c:c + 1])
                        else:
                            nc.vector.tensor_scalar_mul(
                                outSB_v[u][:, I * 4 + c, h, :],
                                av_v[:, c, 0:64], rl[:, c:c + 1])
                # O [q, dh] -> O^T [dh, q] (PE transpose via av ring slots);
                # for unit 1 the projection of each q-chunk follows its
                # transpose immediately (needs outT of both units).
                for c in range(4):
                    cg = I * 4 + c
                    ot = ps_av.tile([128, 128], FP16, name=f"ot{u}{cg}",
                                    tag="av")
                    nc.tensor.transpose(ot, outSB[u][:, cg * 128:(cg + 1) * 128],
                                        eye128)
                    nc.vector.tensor_copy(outT[u][:, cg * 128:(cg + 1) * 128],
                                          ot)

            def outproj_chunk(qc):
                qs = slice(qc * 128, (qc + 1) * 128)
                ysb = wp.tile([128, 1024], FP16, name=f"ys{qc}", tag="ysb",
                              bufs=4)
                for nck in range(2):
                    ns = slice(nck * 512, (nck + 1) * 512)
                    pool_ = ps_stage if qc >= 8 and nck == 0 else ps_acc
                    yps = pool_.tile([128, 512], F32, name=f"y{qc}{nck}",
                                     tag="stage" if qc >= 8 and nck == 0
                                     else "acc")
                    nc.tensor.matmul(yps, outT[0][:, qs],
                                     wout_v[:, 0, ns], start=True,
                                     stop=False, skip_group_check=True)
                    nc.tensor.matmul(yps, outT[1][:, qs],
                                     wout_v[:, 1, ns], start=False,
                                     stop=True, skip_group_check=True)
                    if qc >= 8 and nck == 0:
                        nc.scalar.activation(ysb[:, ns], yps, COPY)
                    else:
                        nc.vector.tensor_copy(ysb[:, ns], yps)
                nc.sync.dma_start(out=y_d[qs, :], in_=ysb)

            for r in range(NQB):
                qkv_pass(0, r)
                attn_qblock(0, r)
            for r in range(NQB):
                qkv_pass(1, r)
                attn_qblock(1, r)
            for qc in range(16):                # all proj after last attn
                outproj_chunk(qc)

    nc.compile()
    return nc


def _host_inputs(x, w_qkv, w_out):
    """Build per-core input maps (fp16 device payloads)."""
    x = np.asarray(x, dtype=np.float32)
    w_qkv = np.asarray(w_qkv, dtype=np.float32)
    w_out = np.asarray(w_out, dtype=np.float32)

    xts = [np.ascontiguousarray(x[b].T).astype(np.float16) for b in range(B)]

    consts = np.zeros((128, 320), dtype=np.float16)
    consts[:, 0:64] = np.tile(np.eye(64), (2, 1)).astype(np.float16)
    consts[:, 64:192] = np.eye(128, dtype=np.float16)
    kk = np.arange(128)[:, None]
    qq = np.arange(128)[None, :]
    consts[:, 192:320] = (kk <= qq).astype(np.float16)

    in_maps = []
    for c in range(NCORES):
        b, g = divmod(c, 4)
        heads = [4 * g + i for i in range(4)]
        # wqkv_local: per unit u: [q(128) | k(128) | v(128)] for heads
        # (4g+2u, 4g+2u+1)
        cols = []
        for u in range(2):
            h0, h1 = heads[2 * u], heads[2 * u + 1]
            for part in range(3):  # q, k, v sections at offsets 0, D, 2D
                off = part * D
                cols.append(w_qkv[:, off + h0 * DH: off + (h0 + 1) * DH])
                cols.append(w_qkv[:, off + h1 * DH: off + (h1 + 1) * DH])
        wqkv_local = np.ascontiguousarray(
            np.concatenate(cols, axis=1)).astype(np.float16)
        # wout_local[u]: rows for heads (4g+2u, 4g+2u+1) stacked [64+64, 1024]
        wo = np.zeros((2, 128, 1024), dtype=np.float16)
        for u in range(2):
            h0, h1 = heads[2 * u], heads[2 * u + 1]
            wo[u, 0:64] = w_out[h0 * DH:(h0 + 1) * DH, :]
            wo[u, 64:128] = w_out[h1 * DH:(h1 + 1) * DH, :]
        in_maps.append({
            "xt": xts[b],
            "wqkv": wqkv_local,
            "wout": wo,
            "consts": consts,
        })
    return in_maps


def kernel(x, w_qkv, w_out):
    global LAST_RESULT
    if "nc" not in _CACHE:
        _CACHE["nc"] = _build()
    nc = _CACHE["nc"]
    in_maps = _host_inputs(x, w_qkv, w_out)
    res = run_bass_kernel_spmd(nc, in_maps, list(range(NCORES)))
    LAST_RESULT = res
    y = np.zeros((B, L, D), dtype=np.float32)
    for c in range(NCORES):
        y[c // 4] += res.results[c]["y"].astype(np.float32)
    return y
